# revision 1
# baseline (speedup 1.0000x reference)
"""Trainium2 Bass kernel for nn_AttentionSpatial (manifold attention), v2.

Pipeline (per the reference):
  qkv = 1x1 conv -> 3x3 depthwise conv -> patchify -> per-(b,head,c) unit:
  normalize q,k -> attn = softmax(q k^T * temp) -> cov -> eigh (top-100)
  -> A = U (w_fr^T w_fr) U^T -> out = A v -> re-patchify -> 1x1 conv out.

Sharding: 32 units = (batch 2) x (channel 16); core k=(b,g) owns channels
4g..4g+4 of batch b.  Eigh runs on host (jax CPU f64) because the final
output depends on LAPACK eigenvector signs; everything else on device.

v2 changes vs baseline:
  - 1x1 conv + depthwise fused into one 9-tap PE conv (block-diag weights)
  - f32 PE-transpose (exact; replaces 10-op bf16 3-plane emulation)
  - softmax via ScalarE table Exp (no max-subtraction; |attn|<=1)
  - cov written raw (trace-norm + eps*I dropped: eigvecs invariant)
  - patchify q,k via on-chip SBUF->SBUF DMA split over Sync/Scalar/GpSimd
  - v patchify + P2 permute moved to host between launches (pure data
    movement, like the baseline's host gathers); all L2/L3 DMA contiguous
"""

import numpy as np

PATCH = 16
HEADS = 4
TOP_P = 100
B, C = 2, 16
NCORES = 8

_built = {}
PROFILE = False
LAST_PROFILE = []
DEBUG = {}


def _new_nc():
    from concourse import bacc
    return bacc.Bacc("TRN2", target_bir_lowering=False, debug=False)


# --------------------------------------------------------------------------
# L1: fused 9-tap conv + patchify + attention + cov
# --------------------------------------------------------------------------

def _build_p1():
    import concourse.bass as bass
    import concourse.tile as tile
    from concourse import mybir
    from concourse.masks import make_identity

    f32 = mybir.dt.float32
    f32r = mybir.dt.float32r
    AF = mybir.ActivationFunctionType
    OP = mybir.AluOpType
    nc = _new_nc()

    xb_d = nc.dram_tensor("xb", (16, 258, 258), f32, kind="ExternalInput")
    wt_d = nc.dram_tensor("wt", (128, 9 * 96), f32, kind="ExternalInput")
    tmp_d = nc.dram_tensor("tempu", (1, 1), f32, kind="ExternalInput")
    vimg_d = nc.dram_tensor("vimg", (4, 256, 256), f32, kind="ExternalOutput")
    cov_d = nc.dram_tensor("cov", (4, 256, 256), f32, kind="ExternalOutput")
    qkA_d = nc.dram_tensor("qkA", (8, 8, 16, 256), f32, kind="Internal")
    qkB_d = nc.dram_tensor("qkB", (8, 8, 16, 256), f32, kind="Internal")

    with tile.TileContext(nc) as tc:
        with (
            tc.tile_pool(name="big", bufs=1) as big,
            tc.tile_pool(name="pdp", bufs=1) as pdp,
            tc.tile_pool(name="unit", bufs=4) as up,
            tc.tile_pool(name="small", bufs=3) as sp,
            tc.tile_pool(name="psC", bufs=4, space="PSUM") as psC,
            tc.tile_pool(name="psT", bufs=2, space="PSUM") as psT,
            tc.tile_pool(name="psA", bufs=2, space="PSUM") as psA,
        ):
            # x in four chained slabs: slab q covers out-rows 8q..8q+8
            # (x rows 8q..8q+10).  The tiny marker copies serialize the
            # transfers so slab 0 lands at full bandwidth and the conv
            # pipeline starts ~35us earlier; later slabs stream ahead of
            # the conv's consumption rate.
            SLABS = ((0, 8), (6, 16), (14, 24), (22, 34))
            xq = []
            for q, (r0, r1) in enumerate(SLABS):
                t_xq = big.tile([128, r1 - r0, 258], f32, tag=f"xq{q}")
                xq.append(t_xq)
            for q, (r0, r1) in enumerate(SLABS):
                if q > 0:
                    nc.vector.tensor_copy(xq[q][0:1, 0, 0:1],
                                          xq[q - 1][0:1, 0, 0:1])
                nc.sync.dma_start(
                    xq[q],
                    bass.AP(tensor=xb_d.ap().tensor, offset=r0 * 258,
                            ap=[[32 * 258, 8], [258 * 258, 16],
                                [1, (r1 - r0) * 258]]),
                )
            wt = big.tile([128, 9, 96], f32, tag="wt")
            nc.sync.dma_start(wt.rearrange("p a b -> p (a b)"), wt_d.ap())

            ident = big.tile([128, 128], f32, tag="ident")
            make_identity(nc, ident)
            tempb = big.tile([128, 1], f32, tag="tempb")
            nc.sync.dma_start(
                tempb,
                bass.AP(tensor=tmp_d.ap().tensor, offset=0,
                        ap=[[0, 128], [1, 1]]),
            )


            # ---- HAM warm-up: dense bf16 matmuls spanning the x load so
            # the PE clock is at 8/8 when the f32 conv starts
            identb = big.tile([128, 128], mybir.dt.bfloat16, tag="identb")
            nc.vector.tensor_copy(identb, ident)
            junk = big.tile([128, 512], mybir.dt.bfloat16, tag="junk")
            nc.vector.memset(junk, 1.0)
            prm = psT.tile([128, 2, 256], f32, tag="tps")
            for i in range(40):
                nc.tensor.matmul(prm.rearrange("p a b -> p (a b)"), identb,
                                 junk, start=(i == 0), stop=(i == 39))

            # ---- fused conv, one image row per chunk (contiguous rhs —
            # 2-row strided rhs APs run at half PE throughput)
            q2all = big.tile([96, 32 * 256], f32, tag="q2all")
            taps = [(dy, dx) for dy in (-1, 0, 1) for dx in (-1, 0, 1)]
            for r in range(32):
                acc = psC.tile([96, 256], f32, tag="conv")
                q = 0 if r < 6 else (1 if r < 14 else (2 if r < 22 else 3))
                r0 = SLABS[q][0]
                for t, (dy, dx) in enumerate(taps):
                    rhs = xq[q][:, r - r0 + 1 + dy, dx + 1:dx + 257]
                    nc.tensor.matmul(acc, wt[:, t, :], rhs,
                                     start=(t == 0), stop=(t == 8))
                if r % 2 == 0:
                    nc.vector.tensor_copy(
                        q2all[:, 256 * r:256 * (r + 1)], acc)
                else:
                    nc.scalar.copy(q2all[:, 256 * r:256 * (r + 1)], acc)
                qk_dst = qkA_d if r < 16 else qkB_d
                (nc.sync if r % 2 == 0 else nc.scalar).dma_start(
                    bass.AP(tensor=qk_dst.ap().tensor,
                            offset=(r % 16) * 256,
                            ap=[[32768, 8], [4096, 8], [1, 256]]),
                    q2all[0:64, 256 * r:256 * (r + 1)])

            # ---- v image out (contiguous)
            nc.sync.dma_start(
                vimg_d.ap().rearrange("o (b r) c -> (o b) (r c)", b=8),
                q2all[64:96, :])

            # ---- patchify q,k: even patch-rows (qkA) can start while the
            # second half of the conv still runs
            O_ORDER = [0, 4, 1, 5, 2, 6, 3, 7]
            pd = {}
            for o in range(8):
                t_pd = pdp.tile([128, 2, 256], f32, tag=f"pd{o}")
                pd[o] = t_pd
            def patchify(o, par, eng):
                qk_src = qkA_d if par == 0 else qkB_d
                for c2 in range(2):
                    for prl in range(par, 8, 2):
                        b = 4 * c2 + (prl - par) // 2
                        eng.dma_start(
                            pd[o][16 * prl:16 * (prl + 1), c2, :],
                            bass.AP(tensor=qk_src.ap().tensor,
                                    offset=o * 32768 + b * 4096,
                                    ap=[[16, 16], [256, 16], [1, 16]]),
                        )

            def normalize(o):
                t_pd = pd[o]
                scr = sp.tile([128, 256], f32, tag="scr")
                nrm2 = sp.tile([128, 2], f32, tag="nrm2")
                for c2 in range(2):
                    nc.scalar.activation(scr, t_pd[:, c2, :], AF.Square,
                                         accum_out=nrm2[:, c2:c2 + 1])
                nc.vector.tensor_scalar_max(nrm2, nrm2, 1e-24)
                srt = sp.tile([128, 2], f32, tag="srt")
                nc.scalar.sqrt(srt, nrm2)
                rin = sp.tile([128, 2], f32, tag="rin")
                nc.vector.reciprocal(rin, srt)
                # one newton step on rsqrt fixes table-sqrt error:
                # r1 = r0*(1.5 - 0.5*n2*r0^2)
                nwt = sp.tile([128, 2], f32, tag="nwt")
                nc.vector.tensor_mul(nwt, rin, rin)
                nc.vector.tensor_mul(nwt, nwt, nrm2)
                nc.vector.tensor_scalar(nwt, nwt, -0.5, 1.5,
                                        op0=OP.mult, op1=OP.add)
                nc.vector.tensor_mul(rin, rin, nwt)
                for c2 in range(2):
                    nc.vector.tensor_scalar_mul(
                        t_pd[:, c2, :], t_pd[:, c2, :], rin[:, c2:c2 + 1])

            # even halves overlap the conv's second half
            even_engs = [nc.scalar, nc.scalar, nc.gpsimd, nc.sync]
            for i, o in enumerate(O_ORDER):
                patchify(o, 0, even_engs[i % 4])

            # odd halves + normalize + q/k transposes interleaved per unit
            # pair so unit 0's chain starts right after the conv instead of
            # behind the whole patchify burst
            qTs, kTs, att_es, rssums, xcs, xcTs = {}, {}, {}, {}, {}, {}
            for u in range(4):
                patchify(u, 1, nc.sync)
                patchify(4 + u, 1, nc.gpsimd)
                normalize(u)
                normalize(4 + u)
                qT = up.tile([128, 2, 256], f32, tag="qT")
                qTs[u] = qT
                kT = up.tile([128, 2, 256], f32, tag="kT")
                kTs[u] = kT
                for ti, (src_t, dst_t) in enumerate(((pd[u], qT),
                                                     (pd[4 + u], kT))):
                    tps = psT.tile([128, 2, 256], f32, tag="tps")
                    for nh in range(2):
                        for dh in range(2):
                            nc.tensor.transpose(
                                tps[:, dh, 128 * nh:128 * (nh + 1)],
                                src_t[:, nh, 128 * dh:128 * (dh + 1)],
                                ident)
                    if ti == 0:
                        nc.vector.tensor_copy(dst_t, tps)
                    else:
                        nc.scalar.copy(dst_t, tps)

            for u in range(4):
                att_e = up.tile([128, 2, 256], f32, tag="att_e")
                att_es[u] = att_e
                rssum = sp.tile([128, 2], f32, tag=f"rssum{u}")
                rssums[u] = rssum
                for nh in range(2):
                    att = psA.tile([128, 256], f32, tag="att")
                    for dh in range(2):
                        nc.tensor.matmul(
                            att, qTs[u][:, dh, 128 * nh:128 * (nh + 1)],
                            kTs[u][:, dh, :], start=(dh == 0),
                            stop=(dh == 1))
                    nc.scalar.activation(att_e[:, nh, :], att, AF.Exp,
                                         scale=tempb[:, 0:1],
                                         accum_out=rssum[:, nh:nh + 1])

            for u in range(4):
                rinv = sp.tile([128, 2], f32, tag=f"rinv{u}")
                nc.vector.reciprocal(rinv, rssums[u])
                xc = up.tile([128, 2, 256], f32, tag="xc")
                xcs[u] = xc
                for nh in range(2):
                    nc.vector.tensor_scalar(
                        xc[:, nh, :], att_es[u][:, nh, :],
                        rinv[:, nh:nh + 1], 1.0 / 256.0,
                        op0=OP.mult, op1=OP.subtract)

            for u in range(4):
                xcT = up.tile([128, 2, 256], f32, tag="xcT")
                xcTs[u] = xcT
                tps2 = psT.tile([128, 2, 256], f32, tag="tps")
                for nh in range(2):
                    for mh in range(2):
                        nc.tensor.transpose(
                            tps2[:, mh, 128 * nh:128 * (nh + 1)],
                            xcs[u][:, nh, 128 * mh:128 * (mh + 1)], ident)
                if u % 2 == 0:
                    nc.vector.tensor_copy(xcT, tps2)
                else:
                    nc.scalar.copy(xcT, tps2)

            for u in range(4):
                xcT = xcTs[u]
                cov_sb = up.tile([128, 2, 256], f32, tag="cov_sb")
                for nh in range(2):
                    cv = psA.tile([128, 256], f32, tag="att")
                    for mh in range(2):
                        nc.tensor.matmul(
                            cv, xcT[:, mh, 128 * nh:128 * (nh + 1)],
                            xcT[:, mh, :], start=(mh == 0), stop=(mh == 1))
                    if nh == 0:
                        nc.vector.tensor_copy(cov_sb[:, nh, :], cv)
                    else:
                        nc.scalar.copy(cov_sb[:, nh, :], cv)
                cov_view = cov_d.ap()[u].rearrange("(c p) m -> p c m",
                                                    p=128)
                for nh in range(2):
                    eng = nc.scalar if (2 * u + nh) % 2 == 0 else nc.sync
                    eng.dma_start(cov_view[:, nh, :], cov_sb[:, nh, :])

    nc.compile()
    return nc


# --------------------------------------------------------------------------
# L2: yT = w_fr U^T, A = Y Y^T, M = A v  (all DMA contiguous)
# --------------------------------------------------------------------------

def _build_p2():
    import concourse.tile as tile
    from concourse import mybir

    f32 = mybir.dt.float32
    f32r = mybir.dt.float32r
    nc = _new_nc()

    ut_d = nc.dram_tensor("ut", (4, 100, 256), f32, kind="ExternalInput")
    vpd_d = nc.dram_tensor("vpd", (4, 128, 512), f32, kind="ExternalInput")
    wfrT_d = nc.dram_tensor("wfrT", (100, 100), f32, kind="ExternalInput")
    m_d = nc.dram_tensor("m", (4, 128, 512), f32, kind="ExternalOutput")

    with tile.TileContext(nc) as tc:
        with (
            tc.tile_pool(name="sb", bufs=1) as sb,
            tc.tile_pool(name="unit", bufs=4) as up,
            tc.tile_pool(name="ps", bufs=2, space="PSUM") as ps,
        ):
            wfrT_f = sb.tile([100, 100], f32, tag="wfrT_f")
            nc.sync.dma_start(wfrT_f, wfrT_d.ap())
            wfrT = sb.tile([100, 100], f32r, tag="wfrT")
            nc.vector.tensor_copy(wfrT, wfrT_f)

            for u in range(4):
                ut_f = up.tile([100, 256], f32, tag="ut_f")
                nc.sync.dma_start(ut_f, ut_d.ap()[u])
                ut = up.tile([100, 256], f32r, tag="ut")
                nc.vector.tensor_copy(ut, ut_f)
                v_f = up.tile([128, 2, 256], f32, tag="v_f")
                nc.sync.dma_start(
                    v_f.rearrange("p a b -> p (a b)"), vpd_d.ap()[u])
                v_pd = up.tile([128, 2, 256], f32r, tag="v_pd")
                nc.scalar.copy(v_pd, v_f)

                yTp = ps.tile([100, 256], f32, tag="yT")
                nc.tensor.matmul(yTp, wfrT, ut, start=True, stop=True)
                yT = up.tile([100, 256], f32r, tag="yTs")
                nc.vector.tensor_copy(yT, yTp)

                a_sb = up.tile([128, 2, 256], f32r, tag="a_sb")
                for nh in range(2):
                    ap_ = ps.tile([128, 256], f32, tag="aps")
                    nc.tensor.matmul(ap_, yT[:, 128 * nh:128 * (nh + 1)],
                                     yT, start=True, stop=True)
                    if nh == 0:
                        nc.scalar.copy(a_sb[:, nh, :], ap_)
                    else:
                        nc.vector.tensor_copy(a_sb[:, nh, :], ap_)

                m_sb = up.tile([128, 2, 256], f32, tag="m_sb")
                for mc in range(2):
                    mp = ps.tile([128, 256], f32, tag="mps")
                    for kc in range(2):
                        nc.tensor.matmul(
                            mp, a_sb[:, kc, 128 * mc:128 * (mc + 1)],
                            v_pd[:, kc, :], start=(kc == 0), stop=(kc == 1))
                    if mc == 0:
                        nc.scalar.copy(m_sb[:, mc, :], mp)
                    else:
                        nc.vector.tensor_copy(m_sb[:, mc, :], mp)
                nc.sync.dma_start(
                    m_d.ap()[u], m_sb.rearrange("p a b -> p (a b)"))

    nc.compile()
    return nc


# --------------------------------------------------------------------------
# L3: out[o] = sum_c w_po[o,c] * mimg[c]   (64 rows per core)
# --------------------------------------------------------------------------

def _build_p3():
    import concourse.bass as bass
    import concourse.tile as tile
    from concourse import mybir

    f32 = mybir.dt.float32
    f32r = mybir.dt.float32r
    nc = _new_nc()

    mq_d = nc.dram_tensor("mq", (64, 4096), f32, kind="ExternalInput")
    wpo_d = nc.dram_tensor("wpobig", (64, 64), f32, kind="ExternalInput")
    outq_d = nc.dram_tensor("outq", (64, 4096), f32, kind="ExternalOutput")

    with tile.TileContext(nc) as tc:
        with (
            tc.tile_pool(name="sb", bufs=1) as sb,
            tc.tile_pool(name="ps", bufs=8, space="PSUM") as ps,
        ):
            wpo_f = sb.tile([64, 64], f32, tag="wpo_f")
            nc.sync.dma_start(wpo_f, wpo_d.ap())
            wpo = sb.tile([64, 64], f32r, tag="wpo")
            nc.vector.tensor_copy(wpo, wpo_f)
            m_f = sb.tile([64, 4096], f32, tag="m_f")
            m_in = sb.tile([64, 4096], f32r, tag="m_in")
            for qtr in range(4):
                sl = slice(1024 * qtr, 1024 * (qtr + 1))
                (nc.sync if qtr % 2 == 0 else nc.scalar).dma_start(
                    m_f[:, sl], mq_d.ap()[:, sl])
                if qtr % 2 == 0:
                    nc.vector.tensor_copy(m_in[:, sl], m_f[:, sl])
                else:
                    nc.scalar.copy(m_in[:, sl], m_f[:, sl])
            o_sb = sb.tile([64, 4096], f32, tag="o_sb")
            for ch in range(8):
                acc = ps.tile([64, 512], f32, tag="acc")
                nc.tensor.matmul(acc, wpo, m_in[:, 512 * ch:512 * (ch + 1)],
                                 start=True, stop=True)
                if ch % 2 == 0:
                    nc.vector.tensor_copy(o_sb[:, 512 * ch:512 * (ch + 1)],
                                          acc)
                else:
                    nc.scalar.copy(o_sb[:, 512 * ch:512 * (ch + 1)], acc)
            nc.sync.dma_start(outq_d.ap(), o_sb)

    nc.compile()
    return nc


# --------------------------------------------------------------------------
# host orchestration
# --------------------------------------------------------------------------

def _get(name):
    if name not in _built:
        _built[name] = {"p1": _build_p1, "p2": _build_p2,
                        "p3": _build_p3}[name]()
    return _built[name]


def _run(name, nc, in_maps):
    from concourse.bass_utils import run_bass_kernel_spmd
    r = run_bass_kernel_spmd(nc, in_maps, core_ids=list(range(NCORES)),
                             trace=PROFILE)
    if PROFILE:
        LAST_PROFILE.append((name, r))
    return r.results


def make_p1_inputs(x, w_qkv, w_dw, temperature):
    ins = []
    wq64 = w_qkv.astype(np.float64)
    wd64 = w_dw.astype(np.float64).reshape(48, 9)
    for k in range(NCORES):
        b, g = divmod(k, 4)
        rows = ([4 * g + u for u in range(4)]
                + [16 + 4 * g + u for u in range(4)]
                + [32 + 4 * g + u for u in range(4)])
        # wt[(band,ci), t, o*8+band] = w_qkv[row_o, ci] * w_dw[row_o, t]
        wt = np.zeros((8, 16, 9, 12, 8), np.float64)
        for o in range(12):
            prod = np.einsum('c,t->tc', wq64[rows[o]], wd64[rows[o]])
            for band in range(8):
                wt[band, :, :, o, band] = prod.T
        xpad = np.zeros((16, 258, 258), np.float32)
        xpad[:, 1:257, 1:257] = x[b]
        ins.append({
            "xb": xpad,
            "wt": np.ascontiguousarray(
                wt.reshape(128, 9 * 96).astype(np.float32)),
            "tempu": np.full((1, 1), temperature[g, 0, 0], np.float32),
        })
    return ins


def _host_eigh(cov_all):
    """cov_all: (32,256,256) f32 -> top-100 eigvecs via jax CPU f64 eigh."""
    import jax
    jax.config.update("jax_enable_x64", True)
    import jax.numpy as jnp
    cpu = jax.devices("cpu")[0]
    with jax.default_device(cpu):
        _, vecs = jnp.linalg.eigh(
            jax.device_put(jnp.asarray(cov_all.astype(np.float64)), cpu))
        U = np.asarray(vecs)[:, :, ::-1][:, :, :TOP_P]
    return U


def _patchify_imgs(img):
    """(..., 256, 256) image -> (..., 128, 2, 256) device patch layout:
    [p=16*prl+pc, c2=pr//8, d=16*dr+dc]."""
    s = img.shape[:-2]
    t = img.reshape(s + (2, 8, 16, 16, 16))          # c2, prl, dr, pc, dc
    t = t.transpose(tuple(range(len(s))) + tuple(
        len(s) + np.array([1, 3, 0, 2, 4])))          # prl, pc, c2, dr, dc
    return t.reshape(s + (128, 2, 256))


def kernel(x, w_qkv, w_dw, temperature, w_fr, w_po):
    x = np.ascontiguousarray(np.asarray(x, dtype=np.float32))
    w_qkv = np.asarray(w_qkv, dtype=np.float32)
    w_dw = np.asarray(w_dw, dtype=np.float32)
    temperature = np.asarray(temperature, dtype=np.float32)
    w_fr = np.asarray(w_fr, dtype=np.float32)
    w_po = np.asarray(w_po, dtype=np.float32)

    # ---- L1
    nc1 = _get("p1")
    res1 = _run("p1", nc1, make_p1_inputs(x, w_qkv, w_dw, temperature))

    # ---- host: eigh + v patchify
    cov_all = np.zeros((B, C, 256, 256), np.float32)
    vimg = np.zeros((B, C, 256, 256), np.float32)
    for k in range(NCORES):
        b, g = divmod(k, 4)
        cov_all[b, 4 * g:4 * g + 4] = res1[k]["cov"]
        vimg[b, 4 * g:4 * g + 4] = res1[k]["vimg"]
    DEBUG["cov_all"] = cov_all
    U = _host_eigh(cov_all.reshape(-1, 256, 256))
    UT = np.ascontiguousarray(
        U.transpose(0, 2, 1).astype(np.float32)).reshape(B, C, TOP_P, 256)
    vpd = _patchify_imgs(vimg).reshape(B, C, 128, 512)

    # ---- L2
    nc2 = _get("p2")
    wfrT = np.ascontiguousarray(w_fr.T)
    in2 = []
    for k in range(NCORES):
        b, g = divmod(k, 4)
        in2.append({
            "ut": np.ascontiguousarray(UT[b, 4 * g:4 * g + 4]),
            "vpd": np.ascontiguousarray(vpd[b, 4 * g:4 * g + 4]),
            "wfrT": wfrT,
        })
    res2 = _run("p2", nc2, in2)

    # ---- host: un-chunk M, P2 permute (second patchify), reshard
    mimg = np.zeros((B, C, 256, 256), np.float32)
    for k in range(NCORES):
        b, g = divmod(k, 4)
        mm = res2[k]["m"].reshape(4, 128, 2, 256).transpose(0, 2, 1, 3)
        mimg[b, 4 * g:4 * g + 4] = mm.reshape(4, 16, 16, 16, 16).transpose(
            0, 1, 3, 2, 4).reshape(4, 256, 256)
    DEBUG["mperm"] = mimg
    wpobig = np.zeros((64, 64), np.float32)
    for nq in range(4):
        wpobig[nq * 16:(nq + 1) * 16, nq * 16:(nq + 1) * 16] = w_po.T

    # ---- L3
    nc3 = _get("p3")
    in3 = []
    for k in range(NCORES):
        b, qr = divmod(k, 4)
        # device layout: [part=(nq4, c16), free=(16 rows, 256)]
        blk = mimg[b, :, 64 * qr:64 * (qr + 1), :].reshape(16, 4, 16, 256)
        in3.append({
            "mq": np.ascontiguousarray(
                blk.transpose(1, 0, 2, 3).reshape(64, 4096)),
            "wpobig": wpobig,
        })
    res3 = _run("p3", nc3, in3)

    out = np.zeros((B, C, 256, 256), np.float32)
    for k in range(NCORES):
        b, qr = divmod(k, 4)
        blk = res3[k]["outq"].reshape(4, 16, 16, 256).transpose(1, 0, 2, 3)
        out[b, :, 64 * qr:64 * (qr + 1), :] = blk.reshape(16, 64, 256)
    return out



# revision 4
# speedup vs baseline: 1.2344x; 1.2344x over previous
"""Trainium2 Bass kernel for nn_AttentionSpatial (manifold attention), v2.

Pipeline (per the reference):
  qkv = 1x1 conv -> 3x3 depthwise conv -> patchify -> per-(b,head,c) unit:
  normalize q,k -> attn = softmax(q k^T * temp) -> cov -> eigh (top-100)
  -> A = U (w_fr^T w_fr) U^T -> out = A v -> re-patchify -> 1x1 conv out.

Sharding: 32 units = (batch 2) x (channel 16); core k=(b,g) owns channels
4g..4g+4 of batch b.  Eigh runs on host (jax CPU f64) because the final
output depends on LAPACK eigenvector signs; everything else on device.

v2 changes vs baseline:
  - 1x1 conv + depthwise fused into one 9-tap PE conv (block-diag weights)
  - f32 PE-transpose (exact; replaces 10-op bf16 3-plane emulation)
  - softmax via ScalarE table Exp (no max-subtraction; |attn|<=1)
  - cov written raw (trace-norm + eps*I dropped: eigvecs invariant)
  - patchify q,k via on-chip SBUF->SBUF DMA split over Sync/Scalar/GpSimd
  - v patchify + P2 permute moved to host between launches (pure data
    movement, like the baseline's host gathers); all L2/L3 DMA contiguous
"""

import numpy as np

PATCH = 16
HEADS = 4
TOP_P = 100
B, C = 2, 16
NCORES = 8

_built = {}
PROFILE = False
LAST_PROFILE = []
DEBUG = {}


def _new_nc():
    from concourse import bacc
    return bacc.Bacc("TRN2", target_bir_lowering=False, debug=False)


# --------------------------------------------------------------------------
# L1: fused 9-tap conv + patchify + attention + cov
# --------------------------------------------------------------------------

def _build_p1():
    import concourse.bass as bass
    import concourse.tile as tile
    from concourse import mybir
    from concourse.masks import make_identity

    f32 = mybir.dt.float32
    f32r = mybir.dt.float32r
    AF = mybir.ActivationFunctionType
    OP = mybir.AluOpType
    nc = _new_nc()

    xb_d = nc.dram_tensor("xb", (16, 258, 258), f32r, kind="ExternalInput")
    wt_d = nc.dram_tensor("wt", (128, 9 * 96), f32r, kind="ExternalInput")
    tmp_d = nc.dram_tensor("tempu", (1, 1), f32, kind="ExternalInput")
    vimg_d = nc.dram_tensor("vimg", (4, 256, 256), f32, kind="ExternalOutput")
    cov_d = nc.dram_tensor("cov", (4, 256, 256), f32, kind="ExternalOutput")
    qkA_d = nc.dram_tensor("qkA", (8, 8, 16, 256), f32, kind="Internal")
    qkB_d = nc.dram_tensor("qkB", (8, 8, 16, 256), f32, kind="Internal")

    with tile.TileContext(nc) as tc:
        with (
            tc.tile_pool(name="big", bufs=1) as big,
            tc.tile_pool(name="pdp", bufs=1) as pdp,
            tc.tile_pool(name="unit", bufs=4) as up,
            tc.tile_pool(name="small", bufs=3) as sp,
            tc.tile_pool(name="psC", bufs=4, space="PSUM") as psC,
            tc.tile_pool(name="psT", bufs=2, space="PSUM") as psT,
            tc.tile_pool(name="psA", bufs=2, space="PSUM") as psA,
        ):
            # x in four chained slabs: slab q covers out-rows 8q..8q+8
            # (x rows 8q..8q+10).  The tiny marker copies serialize the
            # transfers so slab 0 lands at full bandwidth and the conv
            # pipeline starts ~35us earlier; later slabs stream ahead of
            # the conv's consumption rate.
            SLABS = ((0, 8), (6, 16), (14, 24), (22, 34))
            xq = []
            for q, (r0, r1) in enumerate(SLABS):
                t_xq = big.tile([128, r1 - r0, 258], f32r, tag=f"xq{q}")
                xq.append(t_xq)
            for q, (r0, r1) in enumerate(SLABS):
                if q > 0:
                    nc.vector.tensor_copy(xq[q][0:1, 0, 0:1],
                                          xq[q - 1][0:1, 0, 0:1])
                nc.sync.dma_start(
                    xq[q],
                    bass.AP(tensor=xb_d.ap().tensor, offset=r0 * 258,
                            ap=[[32 * 258, 8], [258 * 258, 16],
                                [1, (r1 - r0) * 258]]),
                )
            wt = big.tile([128, 9, 96], f32r, tag="wt")
            nc.sync.dma_start(wt.rearrange("p a b -> p (a b)"), wt_d.ap())

            ident = big.tile([128, 128], f32, tag="ident")
            make_identity(nc, ident)
            tempb = big.tile([128, 1], f32, tag="tempb")
            nc.sync.dma_start(
                tempb,
                bass.AP(tensor=tmp_d.ap().tensor, offset=0,
                        ap=[[0, 128], [1, 1]]),
            )


            # ---- HAM warm-up: dense bf16 matmuls spanning the x load so
            # the PE clock is at 8/8 when the f32 conv starts
            identb = big.tile([128, 128], mybir.dt.bfloat16, tag="identb")
            nc.vector.tensor_copy(identb, ident)
            junk = big.tile([128, 512], mybir.dt.bfloat16, tag="junk")
            nc.vector.memset(junk, 1.0)
            prm = psT.tile([128, 2, 256], f32, tag="tps")
            for i in range(40):
                nc.tensor.matmul(prm.rearrange("p a b -> p (a b)"), identb,
                                 junk, start=(i == 0), stop=(i == 39))

            # ---- fused conv, one image row per chunk (contiguous rhs —
            # 2-row strided rhs APs run at half PE throughput); f32r runs
            # 1 cyc/row at >=256 free vs f32's 4
            q2all = big.tile([96, 32 * 256], f32, tag="q2all")
            taps = [(dy, dx) for dy in (-1, 0, 1) for dx in (-1, 0, 1)]
            for r in range(32):
                acc = psC.tile([96, 256], f32, tag="conv")
                q = 0 if r < 6 else (1 if r < 14 else (2 if r < 22 else 3))
                r0 = SLABS[q][0]
                for t, (dy, dx) in enumerate(taps):
                    rhs = xq[q][:, r - r0 + 1 + dy, dx + 1:dx + 257]
                    nc.tensor.matmul(acc, wt[:, t, :], rhs,
                                     start=(t == 0), stop=(t == 8))
                if r % 2 == 0:
                    nc.vector.tensor_copy(
                        q2all[:, 256 * r:256 * (r + 1)], acc)
                else:
                    nc.scalar.copy(q2all[:, 256 * r:256 * (r + 1)], acc)
                qk_dst = qkA_d if r < 16 else qkB_d
                (nc.sync if r % 2 == 0 else nc.scalar).dma_start(
                    bass.AP(tensor=qk_dst.ap().tensor,
                            offset=(r % 16) * 256,
                            ap=[[32768, 8], [4096, 8], [1, 256]]),
                    q2all[0:64, 256 * r:256 * (r + 1)])

            # ---- v image out (contiguous)
            nc.sync.dma_start(
                vimg_d.ap().rearrange("o (b r) c -> (o b) (r c)", b=8),
                q2all[64:96, :])

            # ---- patchify q,k: even patch-rows (qkA) can start while the
            # second half of the conv still runs
            O_ORDER = [0, 4, 1, 5, 2, 6, 3, 7]
            pd = {}
            for o in range(8):
                t_pd = pdp.tile([128, 2, 256], f32, tag=f"pd{o}")
                pd[o] = t_pd
            def patchify(o, par, eng):
                qk_src = qkA_d if par == 0 else qkB_d
                for c2 in range(2):
                    for prl in range(par, 8, 2):
                        b = 4 * c2 + (prl - par) // 2
                        eng.dma_start(
                            pd[o][16 * prl:16 * (prl + 1), c2, :],
                            bass.AP(tensor=qk_src.ap().tensor,
                                    offset=o * 32768 + b * 4096,
                                    ap=[[16, 16], [256, 16], [1, 16]]),
                        )

            def normalize(o):
                t_pd = pd[o]
                scr = sp.tile([128, 256], f32, tag="scr")
                nrm2 = sp.tile([128, 2], f32, tag="nrm2")
                for c2 in range(2):
                    nc.scalar.activation(scr, t_pd[:, c2, :], AF.Square,
                                         accum_out=nrm2[:, c2:c2 + 1])
                nc.vector.tensor_scalar_max(nrm2, nrm2, 1e-24)
                srt = sp.tile([128, 2], f32, tag="srt")
                nc.scalar.sqrt(srt, nrm2)
                rin = sp.tile([128, 2], f32, tag="rin")
                nc.vector.reciprocal(rin, srt)
                # one newton step on rsqrt fixes table-sqrt error:
                # r1 = r0*(1.5 - 0.5*n2*r0^2)
                nwt = sp.tile([128, 2], f32, tag="nwt")
                nc.vector.tensor_mul(nwt, rin, rin)
                nc.vector.tensor_mul(nwt, nwt, nrm2)
                nc.vector.tensor_scalar(nwt, nwt, -0.5, 1.5,
                                        op0=OP.mult, op1=OP.add)
                nc.vector.tensor_mul(rin, rin, nwt)
                for c2 in range(2):
                    nc.vector.tensor_scalar_mul(
                        t_pd[:, c2, :], t_pd[:, c2, :], rin[:, c2:c2 + 1])

            # even halves overlap the conv's second half
            even_engs = [nc.scalar, nc.scalar, nc.gpsimd, nc.sync]
            for i, o in enumerate(O_ORDER):
                patchify(o, 0, even_engs[i % 4])

            # odd halves + normalize + q/k transposes interleaved per unit
            # pair so unit 0's chain starts right after the conv instead of
            # behind the whole patchify burst
            qTs, kTs, att_es, rssums, xcs, xcTs = {}, {}, {}, {}, {}, {}
            for u in range(4):
                patchify(u, 1, nc.sync)
                patchify(4 + u, 1, nc.gpsimd)
                normalize(u)
                normalize(4 + u)
                qT = up.tile([128, 2, 256], f32r, tag="qT")
                qTs[u] = qT
                kT = up.tile([128, 2, 256], f32r, tag="kT")
                kTs[u] = kT
                for ti, (src_t, dst_t) in enumerate(((pd[u], qT),
                                                     (pd[4 + u], kT))):
                    tps = psT.tile([128, 2, 256], f32, tag="tps")
                    for nh in range(2):
                        for dh in range(2):
                            nc.tensor.transpose(
                                tps[:, dh, 128 * nh:128 * (nh + 1)],
                                src_t[:, nh, 128 * dh:128 * (dh + 1)],
                                ident)
                    if ti == 0:
                        nc.vector.tensor_copy(dst_t, tps)
                    else:
                        nc.scalar.copy(dst_t, tps)

            for u in range(4):
                att_e = up.tile([128, 2, 256], f32, tag="att_e")
                att_es[u] = att_e
                rssum = sp.tile([128, 2], f32, tag=f"rssum{u}")
                rssums[u] = rssum
                for nh in range(2):
                    att = psA.tile([128, 256], f32, tag="att")
                    for dh in range(2):
                        nc.tensor.matmul(
                            att, qTs[u][:, dh, 128 * nh:128 * (nh + 1)],
                            kTs[u][:, dh, :], start=(dh == 0),
                            stop=(dh == 1))
                    nc.scalar.activation(att_e[:, nh, :], att, AF.Exp,
                                         scale=tempb[:, 0:1],
                                         accum_out=rssum[:, nh:nh + 1])

            for u in range(4):
                rinv = sp.tile([128, 2], f32, tag=f"rinv{u}")
                nc.vector.reciprocal(rinv, rssums[u])
                xc = up.tile([128, 2, 256], f32, tag="xc")
                xcs[u] = xc
                for nh in range(2):
                    nc.vector.tensor_scalar(
                        xc[:, nh, :], att_es[u][:, nh, :],
                        rinv[:, nh:nh + 1], 1.0 / 256.0,
                        op0=OP.mult, op1=OP.subtract)

            for u in range(4):
                xcT = up.tile([128, 2, 256], f32r, tag="xcT")
                xcTs[u] = xcT
                tps2 = psT.tile([128, 2, 256], f32, tag="tps")
                for nh in range(2):
                    for mh in range(2):
                        nc.tensor.transpose(
                            tps2[:, mh, 128 * nh:128 * (nh + 1)],
                            xcs[u][:, nh, 128 * mh:128 * (mh + 1)], ident)
                if u % 2 == 0:
                    nc.vector.tensor_copy(xcT, tps2)
                else:
                    nc.scalar.copy(xcT, tps2)

            for u in range(4):
                xcT = xcTs[u]
                cov_sb = up.tile([128, 2, 256], f32, tag="cov_sb")
                for nh in range(2):
                    cv = psA.tile([128, 256], f32, tag="att")
                    for mh in range(2):
                        nc.tensor.matmul(
                            cv, xcT[:, mh, 128 * nh:128 * (nh + 1)],
                            xcT[:, mh, :], start=(mh == 0), stop=(mh == 1))
                    if nh == 0:
                        nc.vector.tensor_copy(cov_sb[:, nh, :], cv)
                    else:
                        nc.scalar.copy(cov_sb[:, nh, :], cv)
                cov_view = cov_d.ap()[u].rearrange("(c p) m -> p c m",
                                                    p=128)
                for nh in range(2):
                    eng = nc.scalar if (2 * u + nh) % 2 == 0 else nc.sync
                    eng.dma_start(cov_view[:, nh, :], cov_sb[:, nh, :])

    nc.compile()
    return nc


# --------------------------------------------------------------------------
# L2: yT = w_fr U^T, A = Y Y^T, M = A v  (all DMA contiguous)
# --------------------------------------------------------------------------

def _build_p2():
    import concourse.tile as tile
    from concourse import mybir

    f32 = mybir.dt.float32
    f32r = mybir.dt.float32r
    nc = _new_nc()

    ut_d = nc.dram_tensor("ut", (4, 100, 256), f32, kind="ExternalInput")
    vpd_d = nc.dram_tensor("vpd", (4, 128, 512), f32, kind="ExternalInput")
    wfrT_d = nc.dram_tensor("wfrT", (100, 100), f32, kind="ExternalInput")
    m_d = nc.dram_tensor("m", (4, 128, 512), f32, kind="ExternalOutput")

    with tile.TileContext(nc) as tc:
        with (
            tc.tile_pool(name="sb", bufs=1) as sb,
            tc.tile_pool(name="unit", bufs=4) as up,
            tc.tile_pool(name="ps", bufs=2, space="PSUM") as ps,
        ):
            wfrT_f = sb.tile([100, 100], f32, tag="wfrT_f")
            nc.sync.dma_start(wfrT_f, wfrT_d.ap())
            wfrT = sb.tile([100, 100], f32r, tag="wfrT")
            nc.vector.tensor_copy(wfrT, wfrT_f)

            for u in range(4):
                ut_f = up.tile([100, 256], f32, tag="ut_f")
                nc.sync.dma_start(ut_f, ut_d.ap()[u])
                ut = up.tile([100, 256], f32r, tag="ut")
                nc.vector.tensor_copy(ut, ut_f)
                v_f = up.tile([128, 2, 256], f32, tag="v_f")
                nc.sync.dma_start(
                    v_f.rearrange("p a b -> p (a b)"), vpd_d.ap()[u])
                v_pd = up.tile([128, 2, 256], f32r, tag="v_pd")
                nc.scalar.copy(v_pd, v_f)

                yTp = ps.tile([100, 256], f32, tag="yT")
                nc.tensor.matmul(yTp, wfrT, ut, start=True, stop=True)
                yT = up.tile([100, 256], f32r, tag="yTs")
                nc.vector.tensor_copy(yT, yTp)

                a_sb = up.tile([128, 2, 256], f32r, tag="a_sb")
                for nh in range(2):
                    ap_ = ps.tile([128, 256], f32, tag="aps")
                    nc.tensor.matmul(ap_, yT[:, 128 * nh:128 * (nh + 1)],
                                     yT, start=True, stop=True)
                    if nh == 0:
                        nc.scalar.copy(a_sb[:, nh, :], ap_)
                    else:
                        nc.vector.tensor_copy(a_sb[:, nh, :], ap_)

                m_sb = up.tile([128, 2, 256], f32, tag="m_sb")
                for mc in range(2):
                    mp = ps.tile([128, 256], f32, tag="mps")
                    for kc in range(2):
                        nc.tensor.matmul(
                            mp, a_sb[:, kc, 128 * mc:128 * (mc + 1)],
                            v_pd[:, kc, :], start=(kc == 0), stop=(kc == 1))
                    if mc == 0:
                        nc.scalar.copy(m_sb[:, mc, :], mp)
                    else:
                        nc.vector.tensor_copy(m_sb[:, mc, :], mp)
                nc.sync.dma_start(
                    m_d.ap()[u], m_sb.rearrange("p a b -> p (a b)"))

    nc.compile()
    return nc


# --------------------------------------------------------------------------
# L3: out[o] = sum_c w_po[o,c] * mimg[c]   (64 rows per core)
# --------------------------------------------------------------------------

def _build_p3():
    import concourse.bass as bass
    import concourse.tile as tile
    from concourse import mybir

    f32 = mybir.dt.float32
    f32r = mybir.dt.float32r
    nc = _new_nc()

    mq_d = nc.dram_tensor("mq", (64, 4096), f32, kind="ExternalInput")
    wpo_d = nc.dram_tensor("wpobig", (64, 64), f32, kind="ExternalInput")
    outq_d = nc.dram_tensor("outq", (64, 4096), f32, kind="ExternalOutput")

    with tile.TileContext(nc) as tc:
        with (
            tc.tile_pool(name="sb", bufs=1) as sb,
            tc.tile_pool(name="ps", bufs=8, space="PSUM") as ps,
        ):
            wpo_f = sb.tile([64, 64], f32, tag="wpo_f")
            nc.sync.dma_start(wpo_f, wpo_d.ap())
            wpo = sb.tile([64, 64], f32r, tag="wpo")
            nc.vector.tensor_copy(wpo, wpo_f)
            m_f = sb.tile([64, 4096], f32, tag="m_f")
            m_in = sb.tile([64, 4096], f32r, tag="m_in")
            for qtr in range(4):
                sl = slice(1024 * qtr, 1024 * (qtr + 1))
                (nc.sync if qtr % 2 == 0 else nc.scalar).dma_start(
                    m_f[:, sl], mq_d.ap()[:, sl])
                if qtr % 2 == 0:
                    nc.vector.tensor_copy(m_in[:, sl], m_f[:, sl])
                else:
                    nc.scalar.copy(m_in[:, sl], m_f[:, sl])
            o_sb = sb.tile([64, 4096], f32, tag="o_sb")
            for ch in range(8):
                acc = ps.tile([64, 512], f32, tag="acc")
                nc.tensor.matmul(acc, wpo, m_in[:, 512 * ch:512 * (ch + 1)],
                                 start=True, stop=True)
                if ch % 2 == 0:
                    nc.vector.tensor_copy(o_sb[:, 512 * ch:512 * (ch + 1)],
                                          acc)
                else:
                    nc.scalar.copy(o_sb[:, 512 * ch:512 * (ch + 1)], acc)
            nc.sync.dma_start(outq_d.ap(), o_sb)

    nc.compile()
    return nc


# --------------------------------------------------------------------------
# host orchestration
# --------------------------------------------------------------------------

def _get(name):
    if name not in _built:
        _built[name] = {"p1": _build_p1, "p2": _build_p2,
                        "p3": _build_p3}[name]()
    return _built[name]


def _run(name, nc, in_maps):
    from concourse.bass_utils import run_bass_kernel_spmd
    r = run_bass_kernel_spmd(nc, in_maps, core_ids=list(range(NCORES)),
                             trace=PROFILE)
    if PROFILE:
        LAST_PROFILE.append((name, r))
    return r.results


def make_p1_inputs(x, w_qkv, w_dw, temperature):
    ins = []
    wq64 = w_qkv.astype(np.float64)
    wd64 = w_dw.astype(np.float64).reshape(48, 9)
    for k in range(NCORES):
        b, g = divmod(k, 4)
        rows = ([4 * g + u for u in range(4)]
                + [16 + 4 * g + u for u in range(4)]
                + [32 + 4 * g + u for u in range(4)])
        # wt[(band,ci), t, o*8+band] = w_qkv[row_o, ci] * w_dw[row_o, t]
        wt = np.zeros((8, 16, 9, 12, 8), np.float64)
        for o in range(12):
            prod = np.einsum('c,t->tc', wq64[rows[o]], wd64[rows[o]])
            for band in range(8):
                wt[band, :, :, o, band] = prod.T
        xpad = np.zeros((16, 258, 258), np.float32)
        xpad[:, 1:257, 1:257] = x[b]
        ins.append({
            "xb": xpad,
            "wt": np.ascontiguousarray(
                wt.reshape(128, 9 * 96).astype(np.float32)),
            "tempu": np.full((1, 1), temperature[g, 0, 0], np.float32),
        })
    return ins


def _host_eigh(cov_all):
    """cov_all: (32,256,256) f32 -> top-100 eigvecs via jax CPU f64 eigh."""
    import jax
    jax.config.update("jax_enable_x64", True)
    import jax.numpy as jnp
    cpu = jax.devices("cpu")[0]
    with jax.default_device(cpu):
        _, vecs = jnp.linalg.eigh(
            jax.device_put(jnp.asarray(cov_all.astype(np.float64)), cpu))
        U = np.asarray(vecs)[:, :, ::-1][:, :, :TOP_P]
    return U


def _patchify_imgs(img):
    """(..., 256, 256) image -> (..., 128, 2, 256) device patch layout:
    [p=16*prl+pc, c2=pr//8, d=16*dr+dc]."""
    s = img.shape[:-2]
    t = img.reshape(s + (2, 8, 16, 16, 16))          # c2, prl, dr, pc, dc
    t = t.transpose(tuple(range(len(s))) + tuple(
        len(s) + np.array([1, 3, 0, 2, 4])))          # prl, pc, c2, dr, dc
    return t.reshape(s + (128, 2, 256))


def kernel(x, w_qkv, w_dw, temperature, w_fr, w_po):
    x = np.ascontiguousarray(np.asarray(x, dtype=np.float32))
    w_qkv = np.asarray(w_qkv, dtype=np.float32)
    w_dw = np.asarray(w_dw, dtype=np.float32)
    temperature = np.asarray(temperature, dtype=np.float32)
    w_fr = np.asarray(w_fr, dtype=np.float32)
    w_po = np.asarray(w_po, dtype=np.float32)

    # ---- L1
    nc1 = _get("p1")
    res1 = _run("p1", nc1, make_p1_inputs(x, w_qkv, w_dw, temperature))

    # ---- host: eigh + v patchify
    cov_all = np.zeros((B, C, 256, 256), np.float32)
    vimg = np.zeros((B, C, 256, 256), np.float32)
    for k in range(NCORES):
        b, g = divmod(k, 4)
        cov_all[b, 4 * g:4 * g + 4] = res1[k]["cov"]
        vimg[b, 4 * g:4 * g + 4] = res1[k]["vimg"]
    DEBUG["cov_all"] = cov_all
    U = _host_eigh(cov_all.reshape(-1, 256, 256))
    UT = np.ascontiguousarray(
        U.transpose(0, 2, 1).astype(np.float32)).reshape(B, C, TOP_P, 256)
    vpd = _patchify_imgs(vimg).reshape(B, C, 128, 512)

    # ---- L2
    nc2 = _get("p2")
    wfrT = np.ascontiguousarray(w_fr.T)
    in2 = []
    for k in range(NCORES):
        b, g = divmod(k, 4)
        in2.append({
            "ut": np.ascontiguousarray(UT[b, 4 * g:4 * g + 4]),
            "vpd": np.ascontiguousarray(vpd[b, 4 * g:4 * g + 4]),
            "wfrT": wfrT,
        })
    res2 = _run("p2", nc2, in2)

    # ---- host: un-chunk M, P2 permute (second patchify), reshard
    mimg = np.zeros((B, C, 256, 256), np.float32)
    for k in range(NCORES):
        b, g = divmod(k, 4)
        mm = res2[k]["m"].reshape(4, 128, 2, 256).transpose(0, 2, 1, 3)
        mimg[b, 4 * g:4 * g + 4] = mm.reshape(4, 16, 16, 16, 16).transpose(
            0, 1, 3, 2, 4).reshape(4, 256, 256)
    DEBUG["mperm"] = mimg
    wpobig = np.zeros((64, 64), np.float32)
    for nq in range(4):
        wpobig[nq * 16:(nq + 1) * 16, nq * 16:(nq + 1) * 16] = w_po.T

    # ---- L3
    nc3 = _get("p3")
    in3 = []
    for k in range(NCORES):
        b, qr = divmod(k, 4)
        # device layout: [part=(nq4, c16), free=(16 rows, 256)]
        blk = mimg[b, :, 64 * qr:64 * (qr + 1), :].reshape(16, 4, 16, 256)
        in3.append({
            "mq": np.ascontiguousarray(
                blk.transpose(1, 0, 2, 3).reshape(64, 4096)),
            "wpobig": wpobig,
        })
    res3 = _run("p3", nc3, in3)

    out = np.zeros((B, C, 256, 256), np.float32)
    for k in range(NCORES):
        b, qr = divmod(k, 4)
        blk = res3[k]["outq"].reshape(4, 16, 16, 256).transpose(1, 0, 2, 3)
        out[b, :, 64 * qr:64 * (qr + 1), :] = blk.reshape(16, 64, 256)
    return out

